# revision 31
# baseline (speedup 1.0000x reference)
"""GAT-style attention kernel for Trainium2, data-parallel over batch on 8 cores.

Computation (per batch i):
    scores = (X @ X^T) / 16                      X = inp[i]  [N, D]
    E      = exp(scores) * adj[i]                [N, N]
    out    = leaky_relu((E / (rowsum(E)+1e-10)) @ (X @ W) + b, 0.01)

Device-side layout (per core, one batch):
    xT   [D, N]       : X transposed (host-prepped) -> SBUF chunk tiles
    adjq [128, N/128, N] fp8e4 : adj^T re-laid so adjq[p, nb, m] = adj[m, nb*128+p]
    W    [D, D]
    out  [N, D] f32

The E tile is produced directly in transposed orientation (n on partitions)
using the symmetry of scores, so the second matmul needs no transposes:
    ET[n, m] = exp(S[n, m]/16) * adjT[n, m]
    U'[m, o] = sum_n ET[n, m] * h'[n, o],  h' = [X@W, 1, 0]  (ones col = denom)

Matmul dtypes: scores in fp32r (11-bit mantissa, full PE rate at N>=256);
attention matmul in bf16 (fast weight load). All accumulation is fp32 PSUM.
"""

import sys

if "/opt/trn_rl_repo" not in sys.path:
    sys.path.insert(0, "/opt/trn_rl_repo")

from contextlib import ExitStack

import ml_dtypes
import numpy as np

import concourse.bacc as bacc
import concourse.tile as tile
from concourse import mybir
from concourse import bass_utils
from concourse.bass import compact_to_ranges
from concourse.tile import ScopedClock

B, N, D = 8, 2048, 256
NCORES = 8

F32 = mybir.dt.float32
F32R = mybir.dt.float32r
BF16 = mybir.dt.bfloat16
FP8 = mybir.dt.float8e4
AF = mybir.ActivationFunctionType
ALU = mybir.AluOpType

S_MODE = "f32r"          # scores / X@W matmul operand dtype: "f32r" | "f32"
E_MODE = "bf16"          # attention matmul operand dtype: "f32r" | "bf16"


class SlimTailTileContext(tile.TileContext):
    """TileContext whose kernel tail is drain + one barrier.

    The stock tail also clears every Tile semaphore (a ~7us EVSEM storm).
    Instead the program clears the whole kernel semaphore window in its own
    preamble (see build_program), which makes the NEFF self-initializing and
    re-executable without the tail clear.
    """

    def _drain_and_barrier(self, tick_clock, wait_clock):
        drain_inst = self.nc.sync.drain()
        wait_clock.add_sem_waits(
            drain_inst.ins, ScopedClock({None: tick_clock.global_clock})
        )
        self.nc.all_engine_barrier(sem_only=True)
        popped = self.nc._tile_sem_poison_stack.pop()
        assert popped is self._sem_poison


def round_fp32r(a):
    """Round fp32 -> fp32r bit pattern (11-bit mantissa, round-half-up)."""
    u = np.ascontiguousarray(a, dtype=np.float32).view(np.uint32)
    r = ((u + 0x800) & np.uint32(0xFFFFF000)).astype(np.uint32)
    return r.view(np.float32)


def build_program(n=N, d=D, mc_size=512, add_bias=False,
                  s_mode=None, e_mode=None):
    """Build the single-core Bass program (SPMD across cores via in_maps)."""
    s_mode = s_mode or S_MODE
    e_mode = e_mode or E_MODE
    SDT = {"f32r": F32R, "f32": F32}[s_mode]
    EDT = {"f32r": F32R, "bf16": BF16, "f32": F32}[e_mode]

    nb_count = n // 128          # partition blocks over nodes
    kb = d // 128                # contraction blocks over feature dim
    nmc = n // mc_size           # m-chunks
    msub = mc_size // 128        # 128-wide m sub-blocks per chunk

    nc = bacc.Bacc("TRN2", target_bir_lowering=False, debug=False,
                   enable_asserts=False)

    # Self-initialize: clear the kernel's semaphore window up front so the
    # slim tail (no end-of-kernel sem clear) stays re-execution safe.
    clear_range = [s for s in nc._kernel_sem_range if s not in nc.barrier_sems]
    for r in compact_to_ranges(clear_range):
        nc.gpsimd.dma_reset(r)
        nc.gpsimd.sem_clear(r)
    nc._nrt_pseudo_barrier()

    xT = nc.dram_tensor("xT", [d, n], SDT, kind="ExternalInput").ap()
    adjq = nc.dram_tensor("adjq", [128, nb_count, n], FP8,
                          kind="ExternalInput").ap()
    Wd = nc.dram_tensor("W", [d, d], SDT, kind="ExternalInput").ap()
    if add_bias:
        bd = nc.dram_tensor("b_bcast", [128, d], F32, kind="ExternalInput").ap()
    outd = nc.dram_tensor("out", [n, d], F32, kind="ExternalOutput").ap()

    with SlimTailTileContext(nc) as tc, ExitStack() as ctx:
        const = ctx.enter_context(tc.tile_pool(name="const", bufs=1))
        work = ctx.enter_context(tc.tile_pool(name="work", bufs=5))
        fin = ctx.enter_context(tc.tile_pool(name="fin", bufs=4))
        spsum = ctx.enter_context(tc.tile_pool(name="spsum", bufs=4, space="PSUM"))
        upsum = ctx.enter_context(tc.tile_pool(name="upsum", bufs=1, space="PSUM"))

        # ---- constants / resident tensors ----
        # DMA supply order is the pipeline-fill critical path.  The first xT
        # chunk rides the scalar-engine HWDGE ring (otherwise empty), the
        # remaining chunks the sync ring, and adj streams via gpsimd SWDGE -
        # three independent descriptor queues supplying the 16 SDMA engines.
        xchunk = min(512, n)
        nxc = n // xchunk
        xtc = [[const.tile([128, xchunk], SDT, tag=f"xt{k}_{c}",
                           name=f"xt{k}_{c}") for c in range(nxc)]
               for k in range(kb)]
        for c in range(nxc):
            for k in range(kb):
                eng = nc.sync
                eng.dma_start(
                    out=xtc[k][c],
                    in_=xT[k * 128:(k + 1) * 128,
                           c * xchunk:(c + 1) * xchunk])
        wt = []
        for k in range(kb):
            w_ = const.tile([128, d], SDT, tag=f"wt{k}", name=f"wt{k}")
            nc.scalar.dma_start(out=w_, in_=Wd[k * 128:(k + 1) * 128, :])
            wt.append(w_)
        if add_bias:
            bt = const.tile([128, d], F32, tag="bt", name="bt")
            nc.sync.dma_start(out=bt, in_=bd[:, :])

        def xt_cols(k, a, b):
            """View of xT[k*128:(k+1)*128, a:b] (must stay in one chunk)."""
            c = a // xchunk
            assert b <= (c + 1) * xchunk
            return xtc[k][c][:, a - c * xchunk:b - c * xchunk]

        adjt = []
        for nb in range(nb_count):
            a_ = const.tile([128, n], FP8, tag=f"adj{nb}", name=f"adj{nb}")
            nc.gpsimd.dma_start(out=a_, in_=adjq[:, nb, :])
            adjt.append(a_)

        # cols d:d+2 of h' are [1, 0]; fp32r matmul needs an even free count
        ones_c = const.tile([128, 2], F32, tag="ones", name="ones_c")
        nc.vector.memset(ones_c[:, 0:1], 1.0)
        nc.vector.memset(ones_c[:, 1:2], 0.0)
        # h' tiles are produced inside the first m-chunk pass (below) so the
        # PE warms up on them while adj/xT still stream in.
        hp = [None] * nb_count

        # ---- finalize: denom divide (ACT), leaky relu (DVE), DMA out ----
        def emit_finalize(uts, mc):
            for j in range(msub):
                mb_idx = mc * msub + j
                rec = fin.tile([128, 1], F32, tag="rec", name=f"rec{mb_idx}")
                nc.vector.reciprocal(rec, uts[j][:, d:d + 1])
                ot = fin.tile([128, d], F32, tag="ot", name=f"ot{mb_idx}")
                # divide by denom, evacuate PSUM on ACT
                nc.scalar.activation(ot, uts[j][:, 0:d], AF.Copy, scale=rec)
                if add_bias:
                    nc.vector.tensor_add(ot, ot, bt)
                oo = fin.tile([128, d], F32, tag="oo", name=f"oo{mb_idx}")
                # leaky_relu(x) = max(0.01*x, x) in one DVE op
                nc.vector.scalar_tensor_tensor(
                    out=oo, in0=ot, scalar=0.01, in1=ot,
                    op0=ALU.mult, op1=ALU.max,
                )
                nc.sync.dma_start(
                    out=outd[mb_idx * 128:(mb_idx + 1) * 128, :], in_=oo)

        # ---- main loop: m-chunks outer, n-blocks inner, software-skewed ----
        SKEW = 3   # iterations between producing ET(nb) and its U' matmuls

        def emit_uprime(uts, e2, nb):
            for j in range(msub):
                nc.tensor.matmul(
                    uts[j],
                    lhsT=e2[:, j * 128:(j + 1) * 128],
                    rhs=hp[nb],
                    start=(nb == 0), stop=(nb == nb_count - 1),
                )

        pending_finalize = None
        for mc in range(nmc):
            uts = [upsum.tile([128, d + 2], F32, tag=f"u{j}", name=f"u{mc}_{j}")
                   for j in range(msub)]
            pend = []
            for nb in range(nb_count):
                # scores tile S[n-block, m-chunk] (symmetric: equals S^T slice)
                sp = spsum.tile([128, mc_size], F32, tag="s", name=f"sp{mc}_{nb}")
                for k in range(kb):
                    nc.tensor.matmul(
                        sp,
                        lhsT=xt_cols(k, nb * 128, (nb + 1) * 128),
                        rhs=xt_cols(k, mc * mc_size, (mc + 1) * mc_size),
                        start=(k == 0), stop=(k == kb - 1),
                    )
                if mc == 0:
                    # fold the h' computation into the first m-chunk pass so
                    # the PE warms up while adj/xT still stream in
                    hps = spsum.tile([128, d], F32, tag="s", name=f"hps{nb}")
                    for k in range(kb):
                        nc.tensor.matmul(
                            hps,
                            lhsT=xt_cols(k, nb * 128, (nb + 1) * 128),
                            rhs=wt[k],
                            start=(k == 0), stop=(k == kb - 1),
                        )
                # exp output in the e-matmul dtype (bf16 halves DVE cost)
                et_dt = EDT if e_mode == "bf16" else F32
                et = work.tile([128, mc_size], et_dt, tag="e", name=f"et{mc}_{nb}")
                nc.scalar.activation(et, sp, AF.Exp, scale=1.0 / 16.0)
                e2 = work.tile([128, mc_size], EDT, tag="e2", name=f"e2{mc}_{nb}")
                nc.vector.tensor_mul(
                    e2, et, adjt[nb][:, mc * mc_size:(mc + 1) * mc_size])
                if mc == 0:
                    h_ = const.tile([128, d + 2], EDT, tag=f"h{nb}", name=f"h{nb}")
                    # alternate the PSUM evacuation between ACT and DVE so
                    # neither engine becomes the mc0 bottleneck
                    if nb % 2 == 0:
                        nc.vector.tensor_copy(h_[:, 0:d], hps)
                    else:
                        nc.scalar.activation(h_[:, 0:d], hps, AF.Copy)
                    nc.vector.tensor_copy(h_[:, d:d + 2], ones_c)
                    hp[nb] = h_
                pend.append((e2, nb))
                if len(pend) > SKEW:
                    emit_uprime(uts, *pend.pop(0))
                if nb == 1 and pending_finalize is not None:
                    pending_finalize()
                    pending_finalize = None
            for e2_, pnb in pend:
                emit_uprime(uts, e2_, pnb)
            pending_finalize = (lambda uts=uts, mc=mc: emit_finalize(uts, mc))
        pending_finalize()

    nc.compile()
    return nc


def shard_inputs(inp, adj, W, b, n=N, d=D, add_bias=False, s_mode=None):
    """Host-side prep: per-core input maps."""
    s_mode = s_mode or S_MODE
    inp = np.ascontiguousarray(np.asarray(inp, dtype=np.float32))
    adj = np.asarray(adj, dtype=np.float32)
    W = np.ascontiguousarray(np.asarray(W, dtype=np.float32))
    b = np.asarray(b, dtype=np.float32)
    if s_mode == "f32r":
        W = round_fp32r(W)
    nb_count = n // 128
    in_maps = []
    for i in range(NCORES):
        xT = np.ascontiguousarray(inp[i].T)                       # [d, n]
        if s_mode == "f32r":
            xT = round_fp32r(xT)
        a8 = adj[i].astype(ml_dtypes.float8_e4m3)                 # [m, n]
        adjq = np.ascontiguousarray(
            a8.reshape(n, nb_count, 128).transpose(2, 1, 0))      # [128, nb, m]
        m = {"xT": xT, "adjq": adjq, "W": W}
        if add_bias:
            m["b_bcast"] = np.ascontiguousarray(
                np.broadcast_to(b, (128, d)).astype(np.float32))
        in_maps.append(m)
    return in_maps


_prog_cache = {}


def _get_program(add_bias):
    key = add_bias
    if key not in _prog_cache:
        _prog_cache[key] = build_program(add_bias=add_bias)
    return _prog_cache[key]


def run(inp, adj, W, b, trace=False):
    add_bias = bool(np.any(np.asarray(b) != 0))
    nc = _get_program(add_bias)
    in_maps = shard_inputs(inp, adj, W, b, add_bias=add_bias)
    res = bass_utils.run_bass_kernel_spmd(
        nc, in_maps, core_ids=list(range(NCORES)), trace=trace)
    out = np.stack([res.results[i]["out"] for i in range(NCORES)], axis=0)
    return out.astype(np.float32), res


def kernel(inp, adj, W, b):
    out, _ = run(inp, adj, W, b)
    return out


# revision 33
# speedup vs baseline: 1.0257x; 1.0257x over previous
"""GAT-style attention kernel for Trainium2, data-parallel over batch on 8 cores.

Computation (per batch i):
    scores = (X @ X^T) / 16                      X = inp[i]  [N, D]
    E      = exp(scores) * adj[i]                [N, N]
    out    = leaky_relu((E / (rowsum(E)+1e-10)) @ (X @ W) + b, 0.01)

Device-side layout (per core, one batch):
    xT   [D, N]       : X transposed (host-prepped) -> SBUF chunk tiles
    adjq [128, N/128, N] fp8e4 : adj^T re-laid so adjq[p, nb, m] = adj[m, nb*128+p]
    W    [D, D]
    out  [N, D] f32

The E tile is produced directly in transposed orientation (n on partitions)
using the symmetry of scores, so the second matmul needs no transposes:
    ET[n, m] = exp(S[n, m]/16) * adjT[n, m]
    U'[m, o] = sum_n ET[n, m] * h'[n, o],  h' = [X@W, 1, 0]  (ones col = denom)

Matmul dtypes: scores in fp32r (11-bit mantissa, full PE rate at N>=256);
attention matmul in bf16 (fast weight load). All accumulation is fp32 PSUM.
"""

import sys

if "/opt/trn_rl_repo" not in sys.path:
    sys.path.insert(0, "/opt/trn_rl_repo")

from contextlib import ExitStack

import ml_dtypes
import numpy as np

import concourse.bacc as bacc
import concourse.tile as tile
from concourse import mybir
from concourse import bass_utils
from concourse.bass import compact_to_ranges
from concourse.tile import ScopedClock

B, N, D = 8, 2048, 256
NCORES = 8

F32 = mybir.dt.float32
F32R = mybir.dt.float32r
BF16 = mybir.dt.bfloat16
FP8 = mybir.dt.float8e4
AF = mybir.ActivationFunctionType
ALU = mybir.AluOpType

S_MODE = "f32r"          # scores / X@W matmul operand dtype: "f32r" | "f32"
E_MODE = "bf16"          # attention matmul operand dtype: "f32r" | "bf16"


class SlimTailTileContext(tile.TileContext):
    """TileContext whose kernel tail is drain + one barrier.

    The stock tail also clears every Tile semaphore (a ~7us EVSEM storm).
    Instead the program clears the whole kernel semaphore window in its own
    preamble (see build_program), which makes the NEFF self-initializing and
    re-executable without the tail clear.
    """

    def _drain_and_barrier(self, tick_clock, wait_clock):
        drain_inst = self.nc.sync.drain()
        wait_clock.add_sem_waits(
            drain_inst.ins, ScopedClock({None: tick_clock.global_clock})
        )
        self.nc.all_engine_barrier(sem_only=True)
        popped = self.nc._tile_sem_poison_stack.pop()
        assert popped is self._sem_poison


def round_fp32r(a):
    """Round fp32 -> fp32r bit pattern (11-bit mantissa, round-half-up)."""
    u = np.ascontiguousarray(a, dtype=np.float32).view(np.uint32)
    r = ((u + 0x800) & np.uint32(0xFFFFF000)).astype(np.uint32)
    return r.view(np.float32)


def build_program(n=N, d=D, mc_size=512, add_bias=False,
                  s_mode=None, e_mode=None):
    """Build the single-core Bass program (SPMD across cores via in_maps)."""
    s_mode = s_mode or S_MODE
    e_mode = e_mode or E_MODE
    SDT = {"f32r": F32R, "f32": F32}[s_mode]
    EDT = {"f32r": F32R, "bf16": BF16, "f32": F32}[e_mode]

    nb_count = n // 128          # partition blocks over nodes
    kb = d // 128                # contraction blocks over feature dim
    nmc = n // mc_size           # m-chunks
    msub = mc_size // 128        # 128-wide m sub-blocks per chunk

    nc = bacc.Bacc("TRN2", target_bir_lowering=False, debug=False,
                   enable_asserts=False)

    # Self-initialize: clear the kernel's semaphore window up front so the
    # slim tail (no end-of-kernel sem clear) stays re-execution safe.
    clear_range = [s for s in nc._kernel_sem_range if s not in nc.barrier_sems]
    for r in compact_to_ranges(clear_range):
        nc.gpsimd.dma_reset(r)
        nc.gpsimd.sem_clear(r)
    nc._nrt_pseudo_barrier()

    xT = nc.dram_tensor("xT", [d, n], SDT, kind="ExternalInput").ap()
    adjq = nc.dram_tensor("adjq", [128, nb_count, n], FP8,
                          kind="ExternalInput").ap()
    Wd = nc.dram_tensor("W", [d, d], SDT, kind="ExternalInput").ap()
    if add_bias:
        bd = nc.dram_tensor("b_bcast", [128, d], F32, kind="ExternalInput").ap()
    outd = nc.dram_tensor("out", [n, d], F32, kind="ExternalOutput").ap()

    with SlimTailTileContext(nc) as tc, ExitStack() as ctx:
        const = ctx.enter_context(tc.tile_pool(name="const", bufs=1))
        work = ctx.enter_context(tc.tile_pool(name="work", bufs=5))
        fin = ctx.enter_context(tc.tile_pool(name="fin", bufs=4))
        spsum = ctx.enter_context(tc.tile_pool(name="spsum", bufs=4, space="PSUM"))
        upsum = ctx.enter_context(tc.tile_pool(name="upsum", bufs=1, space="PSUM"))

        # ---- constants / resident tensors ----
        # DMA supply order is the pipeline-fill critical path.  The first xT
        # chunk rides the scalar-engine HWDGE ring (otherwise empty), the
        # remaining chunks the sync ring, and adj streams via gpsimd SWDGE -
        # three independent descriptor queues supplying the 16 SDMA engines.
        xchunk = min(512, n)
        nxc = n // xchunk
        xtc = [[const.tile([128, xchunk], SDT, tag=f"xt{k}_{c}",
                           name=f"xt{k}_{c}") for c in range(nxc)]
               for k in range(kb)]
        for c in range(nxc):
            for k in range(kb):
                eng = nc.sync
                eng.dma_start(
                    out=xtc[k][c],
                    in_=xT[k * 128:(k + 1) * 128,
                           c * xchunk:(c + 1) * xchunk])
        wt = []
        for k in range(kb):
            w_ = const.tile([128, d], SDT, tag=f"wt{k}", name=f"wt{k}")
            nc.scalar.dma_start(out=w_, in_=Wd[k * 128:(k + 1) * 128, :])
            wt.append(w_)
        if add_bias:
            bt = const.tile([128, d], F32, tag="bt", name="bt")
            nc.sync.dma_start(out=bt, in_=bd[:, :])

        def xt_cols(k, a, b):
            """View of xT[k*128:(k+1)*128, a:b] (must stay in one chunk)."""
            c = a // xchunk
            assert b <= (c + 1) * xchunk
            return xtc[k][c][:, a - c * xchunk:b - c * xchunk]

        adjt = []
        for nb in range(nb_count):
            a_ = const.tile([128, n], FP8, tag=f"adj{nb}", name=f"adj{nb}")
            nc.gpsimd.dma_start(out=a_, in_=adjq[:, nb, :])
            adjt.append(a_)

        # cols d:d+2 of h' are [1, 0]; fp32r matmul needs an even free count
        ones_c = const.tile([128, 2], F32, tag="ones", name="ones_c")
        nc.vector.memset(ones_c[:, 0:1], 1.0)
        nc.vector.memset(ones_c[:, 1:2], 0.0)
        # h' tiles are produced inside the first m-chunk pass (below) so the
        # PE warms up on them while adj/xT still stream in.
        hp = [None] * nb_count

        # ---- finalize: denom divide (ACT), leaky relu (DVE), DMA out ----
        def emit_finalize(uts, mc):
            for j in range(msub):
                mb_idx = mc * msub + j
                rec = fin.tile([128, 1], F32, tag="rec", name=f"rec{mb_idx}")
                nc.vector.reciprocal(rec, uts[j][:, d:d + 1])
                ot = fin.tile([128, d], F32, tag="ot", name=f"ot{mb_idx}")
                # divide by denom, evacuate PSUM on ACT
                nc.scalar.activation(ot, uts[j][:, 0:d], AF.Copy, scale=rec)
                if add_bias:
                    nc.vector.tensor_add(ot, ot, bt)
                oo = fin.tile([128, d], F32, tag="oo", name=f"oo{mb_idx}")
                # leaky_relu(x) = max(0.01*x, x) in one DVE op
                nc.vector.scalar_tensor_tensor(
                    out=oo, in0=ot, scalar=0.01, in1=ot,
                    op0=ALU.mult, op1=ALU.max,
                )
                nc.sync.dma_start(
                    out=outd[mb_idx * 128:(mb_idx + 1) * 128, :], in_=oo)

        # ---- main loop: m-chunks outer, n-blocks inner, software-skewed ----
        SKEW = 3   # iterations between producing ET(nb) and its U' matmuls

        def emit_uprime(uts, e2, nb):
            for j in range(msub):
                nc.tensor.matmul(
                    uts[j],
                    lhsT=e2[:, j * 128:(j + 1) * 128],
                    rhs=hp[nb],
                    start=(nb == 0), stop=(nb == nb_count - 1),
                )

        pending_finalize = None
        for mc in range(nmc):
            uts = [upsum.tile([128, d + 2], F32, tag=f"u{j}", name=f"u{mc}_{j}")
                   for j in range(msub)]
            pend = []
            for nb in range(nb_count):
                # scores tile S[n-block, m-chunk] (symmetric: equals S^T slice)
                sp = spsum.tile([128, mc_size], F32, tag="s", name=f"sp{mc}_{nb}")
                for k in range(kb):
                    nc.tensor.matmul(
                        sp,
                        lhsT=xt_cols(k, nb * 128, (nb + 1) * 128),
                        rhs=xt_cols(k, mc * mc_size, (mc + 1) * mc_size),
                        start=(k == 0), stop=(k == kb - 1),
                    )
                if mc == 0:
                    # fold the h' computation into the first m-chunk pass so
                    # the PE warms up while adj/xT still stream in
                    hps = spsum.tile([128, d], F32, tag="s", name=f"hps{nb}")
                    for k in range(kb):
                        nc.tensor.matmul(
                            hps,
                            lhsT=xt_cols(k, nb * 128, (nb + 1) * 128),
                            rhs=wt[k],
                            start=(k == 0), stop=(k == kb - 1),
                        )
                # exp output in the e-matmul dtype (bf16 halves DVE cost)
                et_dt = EDT if e_mode == "bf16" else F32
                et = work.tile([128, mc_size], et_dt, tag="e", name=f"et{mc}_{nb}")
                nc.scalar.activation(et, sp, AF.Exp, scale=1.0 / 16.0)
                e2 = work.tile([128, mc_size], EDT, tag="e2", name=f"e2{mc}_{nb}")
                nc.vector.tensor_mul(
                    e2, et, adjt[nb][:, mc * mc_size:(mc + 1) * mc_size])
                if mc == 0:
                    h_ = const.tile([128, d + 2], EDT, tag=f"h{nb}", name=f"h{nb}")
                    # alternate the PSUM evacuation between ACT and DVE so
                    # neither engine becomes the mc0 bottleneck
                    if nb % 2 == 0:
                        nc.vector.tensor_copy(h_[:, 0:d], hps)
                    else:
                        nc.scalar.activation(h_[:, 0:d], hps, AF.Copy)
                    nc.vector.tensor_copy(h_[:, d:d + 2], ones_c)
                    hp[nb] = h_
                pend.append((e2, nb))
                if len(pend) > SKEW:
                    emit_uprime(uts, *pend.pop(0))
                if nb == 1 and pending_finalize is not None:
                    pending_finalize()
                    pending_finalize = None
            for e2_, pnb in pend:
                emit_uprime(uts, e2_, pnb)
            pending_finalize = (lambda uts=uts, mc=mc: emit_finalize(uts, mc))
        pending_finalize()

    nc.compile()
    return nc


def shard_inputs(inp, adj, W, b, n=N, d=D, add_bias=False, s_mode=None):
    """Host-side prep: per-core input maps."""
    s_mode = s_mode or S_MODE
    inp = np.ascontiguousarray(np.asarray(inp, dtype=np.float32))
    adj = np.asarray(adj, dtype=np.float32)
    W = np.ascontiguousarray(np.asarray(W, dtype=np.float32))
    b = np.asarray(b, dtype=np.float32)
    if s_mode == "f32r":
        W = round_fp32r(W)
    nb_count = n // 128
    in_maps = []
    for i in range(NCORES):
        xT = np.ascontiguousarray(inp[i].T)                       # [d, n]
        if s_mode == "f32r":
            xT = round_fp32r(xT)
        a8 = adj[i].astype(ml_dtypes.float8_e4m3)                 # [m, n]
        adjq = np.ascontiguousarray(
            a8.reshape(n, nb_count, 128).transpose(2, 1, 0))      # [128, nb, m]
        m = {"xT": xT, "adjq": adjq, "W": W}
        if add_bias:
            m["b_bcast"] = np.ascontiguousarray(
                np.broadcast_to(b, (128, d)).astype(np.float32))
        in_maps.append(m)
    return in_maps


_prog_cache = {}


def _get_program(add_bias):
    key = add_bias
    if key not in _prog_cache:
        _prog_cache[key] = build_program(add_bias=add_bias)
    return _prog_cache[key]


def run(inp, adj, W, b, trace=False):
    add_bias = bool(np.any(np.asarray(b) != 0))
    nc = _get_program(add_bias)
    in_maps = shard_inputs(inp, adj, W, b, add_bias=add_bias)
    res = bass_utils.run_bass_kernel_spmd(
        nc, in_maps, core_ids=list(range(NCORES)), trace=trace)
    out = np.stack([res.results[i]["out"] for i in range(NCORES)], axis=0)
    return out.astype(np.float32), res


def kernel(inp, adj, W, b):
    out, _ = run(inp, adj, W, b)
    return out
